# revision 14
# baseline (speedup 1.0000x reference)
"""ADMM-Net (12 unrolled stages) on 8 TRN2 NeuronCores.

Data-parallel over the batch: each core owns a [N, B/8] column block of the
transposed problem. Per stage s (with state u, r where r = yT + rho*(z-u)):

    psum = W_s.T @ r            W_s = (diag(M3_s) M2_s).T, pre-folded on host
    t    = psum + u             (alpha == 1 fast path)
    u'   = clip(t, +-lam/rho)   == t - softthresh(t) ; becomes next u
    d    = t - 2u'
    r'   = yT + rho*d           next stage matmul rhs
    z    = t - u'               (emitted only at the last stage)

Matmuls run in fp16 (both operands): 528 cycles per [128x128]x[128x512]
instruction vs 666 for float32r (fp16 overlaps the PE weight load with
compute; f32r's 4-byte self-load serializes). psum accumulates fp32.

All elementwise state (t, u, d, r, y) is fp16: 16-bit DVE/GpSimd ops run
~2x faster than fp32, and the elementwise chain - not the PE - was the
binding constraint at fp32 (DVE 89% busy vs PE 81%). Ops are split
between DVE and GpSimd to keep both under the PE's per-stage time.
Final rel err ~1.2e-3 (sim) vs the 2e-2 gate.
"""

import os
import numpy as np

S, B, N = 12, 8192, 1024
NCORES = 8
BC = B // NCORES          # batch columns per core
P = 128                   # partitions
KT = N // P               # 8 contraction tiles
MT = N // P               # 8 output-row tiles
CHUNK = 512               # psum bank width (f32)
NB = BC // CHUNK          # 2 column chunks per core
WPOOL_BUFS = 13
SCRATCH_BUFS = 4
FIN_BUFS = 4
PSUM_BUFS = 8
D_ENG = "v"               # engine for d = t - 2u' (stt: only DVE supports it)
R_SPLIT = 4               # of 16 (m,b) units, how many y-adds go to DVE (rest Pool)
LAG = 1                   # m-tiles of software-pipelining lag for the wide chains

_compiled: dict = {}
LAST_RESULT = None        # BassKernelResults of the most recent run (for test.py)


def _build(lam: tuple, rho: float):
    import concourse.tile as tile
    from concourse import bacc, mybir

    f32 = mybir.dt.float32
    f16 = mybir.dt.float16
    Alu = mybir.AluOpType
    Act = mybir.ActivationFunctionType

    nc = bacc.Bacc("TRN2", target_bir_lowering=False, debug=False, num_devices=NCORES)
    yt16_d = nc.dram_tensor("yt16", [N, BC], f16, kind="ExternalInput").ap()
    w_d = nc.dram_tensor("w", [S, KT, P, N], f16, kind="ExternalInput").ap()
    out_d = nc.dram_tensor("out", [N, BC], f32, kind="ExternalOutput").ap()

    def eng(which, m):
        if which == "v":
            return nc.vector
        return nc.gpsimd

    with tile.TileContext(nc) as tc:
        with (
            tc.tile_pool(name="state", bufs=1) as state,
            tc.tile_pool(name="wpool", bufs=WPOOL_BUFS) as wpool,
            tc.tile_pool(name="scratch", bufs=SCRATCH_BUFS) as scratch,
            tc.tile_pool(name="fin", bufs=FIN_BUFS) as fin,
            tc.tile_pool(name="psum", bufs=PSUM_BUFS, space="PSUM") as psum_pool,
        ):
            Y16 = [state.tile([P, BC], f16, tag=f"Y16_{i}", name=f"Y16_{i}") for i in range(KT)]
            for i in range(KT):
                nc.sync.dma_start(Y16[i][:], yt16_d[i * P : (i + 1) * P, :])
            ut = [state.tile([P, BC], f16, tag=f"u{i}", name=f"u{i}") for i in range(MT)]
            rt = [
                [state.tile([P, BC], f16, tag=f"r{p}_{i}", name=f"r{p}_{i}") for i in range(KT)]
                for p in range(2)
            ]

            for s in range(S):
                lam_s = float(lam[s])
                rhs = Y16 if s == 0 else rt[(s - 1) % 2]
                rnew = rt[s % 2]

                slabs = [wpool.tile([P, N], f16, tag="w", name=f"w{s}_{i}") for i in range(KT)]
                for kk in range(KT):
                    nc.sync.dma_start(slabs[kk][:], w_d[s, kk, :, :])

                # x[m]: psum evictions (scalar engine), full BC width per m
                xs = [
                    scratch.tile([P, BC], f32, tag=f"x{m}", name=f"x{s}_{m}", bufs=1)
                    for m in range(MT)
                ]
                pending = []

                def wide_chain(m):
                    """1024-wide elementwise chain for m-tile (both b chunks)."""
                    if s == 0:
                        t_t = xs[m]          # u == 0: t = x
                    else:
                        t_t = scratch.tile([P, BC], f32, tag=f"t{m}", name=f"t{s}_{m}", bufs=1)
                        nc.vector.tensor_tensor(t_t[:], xs[m][:], ut[m][:], Alu.add)
                    if s == S - 1:
                        uc = scratch.tile([P, BC], f32, tag=f"e{m}", name=f"uc{m}", bufs=1)
                        nc.gpsimd.tensor_scalar(uc[:], t_t[:], lam_s, -lam_s, Alu.min, Alu.max)
                        z_t = scratch.tile([P, BC], f32, tag=f"x{m}", name=f"z{m}", bufs=1)
                        nc.vector.tensor_tensor(z_t[:], t_t[:], uc[:], Alu.subtract)
                        nc.sync.dma_start(out_d[m * P : (m + 1) * P, :], z_t[:])
                        return
                    nc.gpsimd.tensor_scalar(ut[m][:], t_t[:], lam_s, -lam_s, Alu.min, Alu.max)
                    e_t = scratch.tile([P, BC], f32, tag=f"e{m}", name=f"e{s}_{m}", bufs=1)
                    e_eng = nc.vector if m % 2 == 0 else nc.gpsimd
                    if rho == 1.0:
                        e_eng.tensor_tensor(e_t[:], t_t[:], Y16[m][:], Alu.add)
                    else:
                        nc.vector.scalar_tensor_tensor(
                            e_t[:], t_t[:], rho, Y16[m][:], Alu.mult, Alu.add
                        )
                    nc.vector.scalar_tensor_tensor(
                        rnew[m][:], ut[m][:], -2.0 * rho, e_t[:], Alu.mult, Alu.add
                    )

                for b in range(NB):
                    bs = slice(b * CHUNK, (b + 1) * CHUNK)
                    for m in range(MT):
                        ps = psum_pool.tile([P, CHUNK], f32, tag="ps", name=f"ps{s}_{m}_{b}")
                        for k in range(KT):
                            nc.tensor.matmul(
                                ps[:],
                                slabs[k][:, m * P : m * P + P],
                                rhs[k][:, bs],
                                start=(k == 0),
                                stop=(k == KT - 1),
                            )
                        nc.scalar.activation(xs[m][:, bs], ps[:], Act.Copy)
                        if b == NB - 1:
                            pending.append(m)
                            if len(pending) > LAG:
                                wide_chain(pending.pop(0))
                for m in pending:
                    wide_chain(m)

    nc.compile()
    return nc


def _pack_weights(M2, M3):
    W = np.transpose(M2 * M3[:, :, None], (0, 2, 1)).astype(np.float32)  # [S,N,N] lhsT
    Wp = W.reshape(S, KT, P, N).astype(np.float16)
    return np.ascontiguousarray(Wp)


def _numpy_fallback(y, M2, M3, alpha, lamb, rho):
    yT = y.T.astype(np.float32)
    z = np.zeros_like(yT)
    u = np.zeros_like(yT)
    for s in range(M2.shape[0]):
        x = M3[s][:, None] * (M2[s] @ (yT + rho * (z - u)))
        x1 = alpha[s] * x + (1.0 - alpha[s]) * z
        v = x1 + u
        t = lamb[s] / rho
        z = np.sign(v) * np.maximum(np.abs(v) - t, 0.0)
        u = v - z
    return np.ascontiguousarray(z.T)


def kernel(y, M2, M3, alpha, lamb, rho):
    global LAST_RESULT
    y = np.asarray(y, dtype=np.float32)
    M2 = np.asarray(M2, dtype=np.float32)
    M3 = np.asarray(M3, dtype=np.float32)
    alpha = np.asarray(alpha, dtype=np.float32)
    lamb = np.asarray(lamb, dtype=np.float32)
    rho_f = float(np.asarray(rho))

    if (
        not np.all(alpha == 1.0)
        or y.shape != (B, N)
        or M2.shape != (S, N, N)
        or M3.shape != (S, N)
    ):
        return _numpy_fallback(y, M2, M3, alpha, lamb, rho_f)

    from concourse.bass_utils import run_bass_kernel_spmd

    lam = tuple(float(l) / rho_f for l in lamb)
    key = (lam, rho_f, WPOOL_BUFS, SCRATCH_BUFS, FIN_BUFS, PSUM_BUFS, D_ENG, R_SPLIT, LAG)
    nc = _compiled.get(key)
    if nc is None:
        nc = _build(lam, rho_f)
        _compiled[key] = nc

    Wp = _pack_weights(M2, M3)
    in_maps = []
    for c in range(NCORES):
        yt_c = np.ascontiguousarray(y[c * BC : (c + 1) * BC, :].T).astype(np.float16)
        in_maps.append({"yt16": yt_c, "w": Wp})

    try:
        import antenv.axon_hooks  # noqa: F401
        trace = bool(os.environ.get("BASS_TRACE"))
    except ImportError:
        # No NTFF hook registry in this image: make sure bass_utils never
        # takes the trace path (it would crash importing antenv.axon_hooks).
        os.environ["BASS_NEVER_TRACE"] = "1"
        trace = False
    res = run_bass_kernel_spmd(nc, in_maps, core_ids=list(range(NCORES)), trace=trace)
    LAST_RESULT = res

    out = np.empty((B, N), dtype=np.float32)
    for c in range(NCORES):
        out[c * BC : (c + 1) * BC, :] = res.results[c]["out"].T
    return out


# revision 15
# speedup vs baseline: 1.2252x; 1.2252x over previous
"""ADMM-Net (12 unrolled stages) on 8 TRN2 NeuronCores.

Data-parallel over the batch: each core owns a [N, B/8] column block of the
transposed problem. Per stage s (with state u, r where r = yT + rho*(z-u)):

    psum = W_s.T @ r            W_s = (diag(M3_s) M2_s).T, pre-folded on host
    t    = psum + u             (alpha == 1 fast path)
    u'   = clip(t, +-lam/rho)   == t - softthresh(t) ; becomes next u
    d    = t - 2u'
    r'   = yT + rho*d           next stage matmul rhs
    z    = t - u'               (emitted only at the last stage)

Matmuls run in fp16 (both operands): 528 cycles per [128x128]x[128x512]
instruction vs 666 for float32r (fp16 overlaps the PE weight load with
compute; f32r's 4-byte self-load serializes). psum accumulates fp32.

All elementwise state (t, u, d, r, y) is fp16: 16-bit DVE/GpSimd ops run
~2x faster than fp32, and the elementwise chain - not the PE - was the
binding constraint at fp32 (DVE 89% busy vs PE 81%). Ops are split
between DVE and GpSimd to keep both under the PE's per-stage time.
Final rel err ~1.2e-3 (sim) vs the 2e-2 gate.
"""

import os
import numpy as np

S, B, N = 12, 8192, 1024
NCORES = 8
BC = B // NCORES          # batch columns per core
P = 128                   # partitions
KT = N // P               # 8 contraction tiles
MT = N // P               # 8 output-row tiles
CHUNK = 512               # psum bank width (f32)
NB = BC // CHUNK          # 2 column chunks per core
WPOOL_BUFS = 13
SCRATCH_BUFS = 5
FIN_BUFS = 4
PSUM_BUFS = 8
D_ENG = "v"               # engine for d = t - 2u' (stt: only DVE supports it)
E_SPLIT = 6               # of 16 (m,b) units, how many e-adds go to DVE (rest Pool)
LAG = 2                   # units of software-pipelining lag for e/rnew ops

_compiled: dict = {}
LAST_RESULT = None        # BassKernelResults of the most recent run (for test.py)


def _build(lam: tuple, rho: float):
    import concourse.tile as tile
    from concourse import bacc, mybir

    f32 = mybir.dt.float32
    f16 = mybir.dt.float16
    Alu = mybir.AluOpType
    Act = mybir.ActivationFunctionType

    nc = bacc.Bacc("TRN2", target_bir_lowering=False, debug=False, num_devices=NCORES)
    yt16_d = nc.dram_tensor("yt16", [N, BC], f16, kind="ExternalInput").ap()
    w_d = nc.dram_tensor("w", [S, KT, P, N], f16, kind="ExternalInput").ap()
    out_d = nc.dram_tensor("out", [N, BC], f32, kind="ExternalOutput").ap()

    with tile.TileContext(nc) as tc:
        with (
            tc.tile_pool(name="state", bufs=1) as state,
            tc.tile_pool(name="wpool", bufs=WPOOL_BUFS) as wpool,
            tc.tile_pool(name="scratch", bufs=SCRATCH_BUFS) as scratch,
            tc.tile_pool(name="psum", bufs=PSUM_BUFS, space="PSUM") as psum_pool,
        ):
            Y16 = [state.tile([P, BC], f16, tag=f"Y16_{i}", name=f"Y16_{i}") for i in range(KT)]
            for i in range(KT):
                nc.sync.dma_start(Y16[i][:], yt16_d[i * P : (i + 1) * P, :])
            ut = [state.tile([P, BC], f16, tag=f"u{i}", name=f"u{i}") for i in range(MT)]
            rt = [
                [state.tile([P, BC], f16, tag=f"r{p}_{i}", name=f"r{p}_{i}") for i in range(KT)]
                for p in range(2)
            ]

            for s in range(S):
                lam_s = float(lam[s])
                rhs = Y16 if s == 0 else rt[(s - 1) % 2]
                rnew = rt[s % 2]

                slabs = [wpool.tile([P, N], f16, tag="w", name=f"w{s}_{i}") for i in range(KT)]
                for kk in range(KT):
                    nc.sync.dma_start(slabs[kk][:], w_d[s, kk, :, :])

                pending = []

                def tail_ops(unit, m, bs, t_t):
                    """e-add + rnew stt (or final z), lagged behind t/clip so
                    in-order engine queues never stall on fresh results."""
                    if s == S - 1:
                        z_t = scratch.tile([P, CHUNK], f32, tag="e", name=f"z{m}_{bs.start}")
                        nc.vector.tensor_tensor(z_t[:], t_t[:], ut[m][:, bs], Alu.subtract)
                        nc.sync.dma_start(out_d[m * P : (m + 1) * P, bs], z_t[:])
                        return
                    e_t = scratch.tile([P, CHUNK], f32, tag="e", name=f"e{s}_{m}_{bs.start}")
                    e_eng = nc.vector if (unit * E_SPLIT) % 16 < E_SPLIT else nc.gpsimd
                    if rho == 1.0:
                        e_eng.tensor_tensor(e_t[:], t_t[:], Y16[m][:, bs], Alu.add)
                    else:
                        nc.vector.scalar_tensor_tensor(
                            e_t[:], t_t[:], rho, Y16[m][:, bs], Alu.mult, Alu.add
                        )
                    nc.vector.scalar_tensor_tensor(
                        rnew[m][:, bs], ut[m][:, bs], -2.0 * rho, e_t[:], Alu.mult, Alu.add
                    )

                for b in range(NB):
                    bs = slice(b * CHUNK, (b + 1) * CHUNK)
                    for m in range(MT):
                        ps = psum_pool.tile([P, CHUNK], f32, tag="ps", name=f"ps{s}_{m}_{b}")
                        for k in range(KT):
                            nc.tensor.matmul(
                                ps[:],
                                slabs[k][:, m * P : m * P + P],
                                rhs[k][:, bs],
                                start=(k == 0),
                                stop=(k == KT - 1),
                            )
                        t_t = scratch.tile([P, CHUNK], f32, tag="t", name=f"t{s}_{m}_{b}")
                        if s == 0:
                            # u == 0: t = psum
                            nc.scalar.activation(t_t[:], ps[:], Act.Copy)
                        else:
                            nc.vector.tensor_tensor(t_t[:], ps[:], ut[m][:, bs], Alu.add)
                        # clip -> u' (fp16): final stage writes uc into ut as well
                        nc.gpsimd.tensor_scalar(ut[m][:, bs], t_t[:], lam_s, -lam_s, Alu.min, Alu.max)
                        pending.append((b * MT + m, m, bs, t_t))
                        if len(pending) > LAG:
                            tail_ops(*pending.pop(0))
                for item in pending:
                    tail_ops(*item)

    nc.compile()
    return nc


def _pack_weights(M2, M3):
    W = np.transpose(M2 * M3[:, :, None], (0, 2, 1)).astype(np.float32)  # [S,N,N] lhsT
    Wp = W.reshape(S, KT, P, N).astype(np.float16)
    return np.ascontiguousarray(Wp)


def _numpy_fallback(y, M2, M3, alpha, lamb, rho):
    yT = y.T.astype(np.float32)
    z = np.zeros_like(yT)
    u = np.zeros_like(yT)
    for s in range(M2.shape[0]):
        x = M3[s][:, None] * (M2[s] @ (yT + rho * (z - u)))
        x1 = alpha[s] * x + (1.0 - alpha[s]) * z
        v = x1 + u
        t = lamb[s] / rho
        z = np.sign(v) * np.maximum(np.abs(v) - t, 0.0)
        u = v - z
    return np.ascontiguousarray(z.T)


def kernel(y, M2, M3, alpha, lamb, rho):
    global LAST_RESULT
    y = np.asarray(y, dtype=np.float32)
    M2 = np.asarray(M2, dtype=np.float32)
    M3 = np.asarray(M3, dtype=np.float32)
    alpha = np.asarray(alpha, dtype=np.float32)
    lamb = np.asarray(lamb, dtype=np.float32)
    rho_f = float(np.asarray(rho))

    if (
        not np.all(alpha == 1.0)
        or y.shape != (B, N)
        or M2.shape != (S, N, N)
        or M3.shape != (S, N)
    ):
        return _numpy_fallback(y, M2, M3, alpha, lamb, rho_f)

    from concourse.bass_utils import run_bass_kernel_spmd

    lam = tuple(float(l) / rho_f for l in lamb)
    key = (lam, rho_f, WPOOL_BUFS, SCRATCH_BUFS, FIN_BUFS, PSUM_BUFS, E_SPLIT, LAG)
    nc = _compiled.get(key)
    if nc is None:
        nc = _build(lam, rho_f)
        _compiled[key] = nc

    Wp = _pack_weights(M2, M3)
    in_maps = []
    for c in range(NCORES):
        yt_c = np.ascontiguousarray(y[c * BC : (c + 1) * BC, :].T).astype(np.float16)
        in_maps.append({"yt16": yt_c, "w": Wp})

    try:
        import antenv.axon_hooks  # noqa: F401
        trace = bool(os.environ.get("BASS_TRACE"))
    except ImportError:
        # No NTFF hook registry in this image: make sure bass_utils never
        # takes the trace path (it would crash importing antenv.axon_hooks).
        os.environ["BASS_NEVER_TRACE"] = "1"
        trace = False
    res = run_bass_kernel_spmd(nc, in_maps, core_ids=list(range(NCORES)), trace=trace)
    LAST_RESULT = res

    out = np.empty((B, N), dtype=np.float32)
    for c in range(NCORES):
        out[c * BC : (c + 1) * BC, :] = res.results[c]["out"].T
    return out


# revision 16
# speedup vs baseline: 1.2291x; 1.0031x over previous
"""ADMM-Net (12 unrolled stages) on 8 TRN2 NeuronCores.

Data-parallel over the batch: each core owns a [N, B/8] column block of the
transposed problem. Per stage s (with state u, r where r = yT + rho*(z-u)):

    psum = W_s.T @ r            W_s = (diag(M3_s) M2_s).T, pre-folded on host
    t    = psum + u             (alpha == 1 fast path)
    u'   = clip(t, +-lam/rho)   == t - softthresh(t) ; becomes next u
    d    = t - 2u'
    r'   = yT + rho*d           next stage matmul rhs
    z    = t - u'               (emitted only at the last stage)

Matmuls run in fp16 (both operands): 528 cycles per [128x128]x[128x512]
instruction vs 666 for float32r (fp16 overlaps the PE weight load with
compute; f32r's 4-byte self-load serializes). psum accumulates fp32.

All elementwise state (t, u, d, r, y) is fp16: 16-bit DVE/GpSimd ops run
~2x faster than fp32, and the elementwise chain - not the PE - was the
binding constraint at fp32 (DVE 89% busy vs PE 81%). Ops are split
between DVE and GpSimd to keep both under the PE's per-stage time.
Final rel err ~1.2e-3 (sim) vs the 2e-2 gate.
"""

import os
import numpy as np

S, B, N = 12, 8192, 1024
NCORES = 8
BC = B // NCORES          # batch columns per core
P = 128                   # partitions
KT = N // P               # 8 contraction tiles
MT = N // P               # 8 output-row tiles
CHUNK = 512               # psum bank width (f32)
NB = BC // CHUNK          # 2 column chunks per core
WPOOL_BUFS = 13
SCRATCH_BUFS = 5
FIN_BUFS = 4
PSUM_BUFS = 8
D_ENG = "v"               # engine for d = t - 2u' (stt: only DVE supports it)
E_SPLIT = 5               # of 16 (m,b) units, how many e-adds go to DVE (rest Pool)
LAG = 3                   # units of software-pipelining lag for e/rnew ops

_compiled: dict = {}
LAST_RESULT = None        # BassKernelResults of the most recent run (for test.py)


def _build(lam: tuple, rho: float):
    import concourse.tile as tile
    from concourse import bacc, mybir

    f32 = mybir.dt.float32
    f16 = mybir.dt.float16
    Alu = mybir.AluOpType
    Act = mybir.ActivationFunctionType

    nc = bacc.Bacc("TRN2", target_bir_lowering=False, debug=False, num_devices=NCORES)
    yt16_d = nc.dram_tensor("yt16", [N, BC], f16, kind="ExternalInput").ap()
    w_d = nc.dram_tensor("w", [S, KT, P, N], f16, kind="ExternalInput").ap()
    out_d = nc.dram_tensor("out", [N, BC], f32, kind="ExternalOutput").ap()

    with tile.TileContext(nc) as tc:
        with (
            tc.tile_pool(name="state", bufs=1) as state,
            tc.tile_pool(name="wpool", bufs=WPOOL_BUFS) as wpool,
            tc.tile_pool(name="scratch", bufs=SCRATCH_BUFS) as scratch,
            tc.tile_pool(name="psum", bufs=PSUM_BUFS, space="PSUM") as psum_pool,
        ):
            Y16 = [state.tile([P, BC], f16, tag=f"Y16_{i}", name=f"Y16_{i}") for i in range(KT)]
            for i in range(KT):
                nc.sync.dma_start(Y16[i][:, 0:CHUNK], yt16_d[i * P : (i + 1) * P, 0:CHUNK])
            ut = [state.tile([P, BC], f16, tag=f"u{i}", name=f"u{i}") for i in range(MT)]
            rt = [
                [state.tile([P, BC], f16, tag=f"r{p}_{i}", name=f"r{p}_{i}") for i in range(KT)]
                for p in range(2)
            ]

            for s in range(S):
                lam_s = float(lam[s])
                rhs = Y16 if s == 0 else rt[(s - 1) % 2]
                rnew = rt[s % 2]

                slabs = [wpool.tile([P, N], f16, tag="w", name=f"w{s}_{i}") for i in range(KT)]
                for kk in range(KT):
                    nc.sync.dma_start(slabs[kk][:], w_d[s, kk, :, :])
                if s == 0:
                    for i in range(KT):
                        nc.sync.dma_start(
                            Y16[i][:, CHUNK:BC], yt16_d[i * P : (i + 1) * P, CHUNK:BC]
                        )

                pending = []

                def tail_ops(unit, m, bs, t_t):
                    """e-add + rnew stt (or final z), lagged behind t/clip so
                    in-order engine queues never stall on fresh results."""
                    if s == S - 1:
                        z_t = scratch.tile([P, CHUNK], f32, tag="e", name=f"z{m}_{bs.start}")
                        zeng = nc.vector if unit % 2 == 0 else nc.gpsimd
                        zeng.tensor_tensor(z_t[:], t_t[:], ut[m][:, bs], Alu.subtract)
                        nc.sync.dma_start(out_d[m * P : (m + 1) * P, bs], z_t[:])
                        return
                    e_t = scratch.tile([P, CHUNK], f32, tag="e", name=f"e{s}_{m}_{bs.start}")
                    e_eng = nc.vector if (unit * E_SPLIT) % 16 < E_SPLIT else nc.gpsimd
                    if rho == 1.0:
                        e_eng.tensor_tensor(e_t[:], t_t[:], Y16[m][:, bs], Alu.add)
                    else:
                        nc.vector.scalar_tensor_tensor(
                            e_t[:], t_t[:], rho, Y16[m][:, bs], Alu.mult, Alu.add
                        )
                    nc.vector.scalar_tensor_tensor(
                        rnew[m][:, bs], ut[m][:, bs], -2.0 * rho, e_t[:], Alu.mult, Alu.add
                    )

                for b in range(NB):
                    bs = slice(b * CHUNK, (b + 1) * CHUNK)
                    for m in range(MT):
                        ps = psum_pool.tile([P, CHUNK], f32, tag="ps", name=f"ps{s}_{m}_{b}")
                        for k in range(KT):
                            nc.tensor.matmul(
                                ps[:],
                                slabs[k][:, m * P : m * P + P],
                                rhs[k][:, bs],
                                start=(k == 0),
                                stop=(k == KT - 1),
                            )
                        t_t = scratch.tile([P, CHUNK], f32, tag="t", name=f"t{s}_{m}_{b}")
                        if s == 0:
                            # u == 0: t = psum
                            nc.scalar.activation(t_t[:], ps[:], Act.Copy)
                        else:
                            nc.vector.tensor_tensor(t_t[:], ps[:], ut[m][:, bs], Alu.add)
                        # clip -> u' (fp16): final stage writes uc into ut as well
                        nc.gpsimd.tensor_scalar(ut[m][:, bs], t_t[:], lam_s, -lam_s, Alu.min, Alu.max)
                        pending.append((b * MT + m, m, bs, t_t))
                        if len(pending) > (0 if s == S - 1 else LAG):
                            tail_ops(*pending.pop(0))
                for item in pending:
                    tail_ops(*item)

    nc.compile()
    return nc


def _pack_weights(M2, M3):
    W = np.transpose(M2 * M3[:, :, None], (0, 2, 1)).astype(np.float32)  # [S,N,N] lhsT
    Wp = W.reshape(S, KT, P, N).astype(np.float16)
    return np.ascontiguousarray(Wp)


def _numpy_fallback(y, M2, M3, alpha, lamb, rho):
    yT = y.T.astype(np.float32)
    z = np.zeros_like(yT)
    u = np.zeros_like(yT)
    for s in range(M2.shape[0]):
        x = M3[s][:, None] * (M2[s] @ (yT + rho * (z - u)))
        x1 = alpha[s] * x + (1.0 - alpha[s]) * z
        v = x1 + u
        t = lamb[s] / rho
        z = np.sign(v) * np.maximum(np.abs(v) - t, 0.0)
        u = v - z
    return np.ascontiguousarray(z.T)


def kernel(y, M2, M3, alpha, lamb, rho):
    global LAST_RESULT
    y = np.asarray(y, dtype=np.float32)
    M2 = np.asarray(M2, dtype=np.float32)
    M3 = np.asarray(M3, dtype=np.float32)
    alpha = np.asarray(alpha, dtype=np.float32)
    lamb = np.asarray(lamb, dtype=np.float32)
    rho_f = float(np.asarray(rho))

    if (
        not np.all(alpha == 1.0)
        or y.shape != (B, N)
        or M2.shape != (S, N, N)
        or M3.shape != (S, N)
    ):
        return _numpy_fallback(y, M2, M3, alpha, lamb, rho_f)

    from concourse.bass_utils import run_bass_kernel_spmd

    lam = tuple(float(l) / rho_f for l in lamb)
    key = (lam, rho_f, WPOOL_BUFS, SCRATCH_BUFS, FIN_BUFS, PSUM_BUFS, E_SPLIT, LAG)
    nc = _compiled.get(key)
    if nc is None:
        nc = _build(lam, rho_f)
        _compiled[key] = nc

    Wp = _pack_weights(M2, M3)
    in_maps = []
    for c in range(NCORES):
        yt_c = np.ascontiguousarray(y[c * BC : (c + 1) * BC, :].T).astype(np.float16)
        in_maps.append({"yt16": yt_c, "w": Wp})

    try:
        import antenv.axon_hooks  # noqa: F401
        trace = bool(os.environ.get("BASS_TRACE"))
    except ImportError:
        # No NTFF hook registry in this image: make sure bass_utils never
        # takes the trace path (it would crash importing antenv.axon_hooks).
        os.environ["BASS_NEVER_TRACE"] = "1"
        trace = False
    res = run_bass_kernel_spmd(nc, in_maps, core_ids=list(range(NCORES)), trace=trace)
    LAST_RESULT = res

    out = np.empty((B, N), dtype=np.float32)
    for c in range(NCORES):
        out[c * BC : (c + 1) * BC, :] = res.results[c]["out"].T
    return out


# revision 19
# speedup vs baseline: 1.4601x; 1.1880x over previous
"""ADMM-Net (12 unrolled stages) on 8 TRN2 NeuronCores.

Data-parallel over the batch: each core owns a [N, B/8] column block of the
transposed problem. Per stage s (with state u, r where r = yT + rho*(z-u)):

    psum = W_s.T @ r            W_s = (diag(M3_s) M2_s).T, pre-folded on host
    t    = psum + u             (alpha == 1 fast path)
    u'   = clip(t, +-lam/rho)   == t - softthresh(t) ; becomes next u
    d    = t - 2u'
    r'   = yT + rho*d           next stage matmul rhs
    z    = t - u'               (emitted only at the last stage)

Matmuls run in fp16 (both operands): 528 cycles per [128x128]x[128x512]
instruction vs 666 for float32r (fp16 overlaps the PE weight load with
compute; f32r's 4-byte self-load serializes). psum accumulates fp32.

All elementwise state (t, u, d, r, y) is fp16: 16-bit DVE/GpSimd ops run
~2x faster than fp32, and the elementwise chain - not the PE - was the
binding constraint at fp32 (DVE 89% busy vs PE 81%). Ops are split
between DVE and GpSimd to keep both under the PE's per-stage time.
Final rel err ~1.2e-3 (sim) vs the 2e-2 gate.
"""

import os
import numpy as np

S, B, N = 12, 8192, 1024
NCORES = 8
BC = B // NCORES          # batch columns per core
P = 128                   # partitions
KT = N // P               # 8 contraction tiles
MT = N // P               # 8 output-row tiles
CHUNK = 512               # psum bank width (f32)
NB = BC // CHUNK          # 2 column chunks per core
WPOOL_BUFS = 13
SCRATCH_BUFS = 5
FIN_BUFS = 4
PSUM_BUFS = 8
D_ENG = "v"               # engine for d = t - 2u' (stt: only DVE supports it)
E_SPLIT = 5               # of 16 (m,b) units, how many e-adds go to DVE (rest Pool)
LAG = 0                   # fused rnew depends only on DVE-local t; no lag needed

_compiled: dict = {}
LAST_RESULT = None        # BassKernelResults of the most recent run (for test.py)



_SOFTSHRINK_ADD = None


def _get_softshrink_add():
    """out = in1 + s1*(in0 - 2*clamp(in0, +-s0)) as one DVE instruction."""
    global _SOFTSHRINK_ADD
    if _SOFTSHRINK_ADD is not None:
        return _SOFTSHRINK_ADD
    from concourse import dve_ops as _dv
    from concourse.dve_spec import Spec, Src0, Src1, C0, C1, minn, maxx, lower
    from concourse.dve_uop import DveOpSpec

    name = "SOFTSHRINK_ADD_ANT"
    if name not in _dv._SUB_OPCODE_FOR_NAME:
        _dv._SUB_OPCODE_FOR_NAME[name] = max(_dv._SUB_OPCODE_FOR_NAME.values()) + 1
        assert _dv._SUB_OPCODE_FOR_NAME[name] < 0x20
    # out = in1 + in0 + s1*clamp(in0, +-s0); call with s1 = -2*rho (rho==1)
    body = Src1 + Src0 + C1 * maxx(minn(Src0, C0), -C0)
    spec = Spec(
        body=body,
        reference=lambda in0, in1, s0, s1, imm2: (
            in1 + in0 + s1 * np.clip(in0, -s0, s0)
        ).astype(np.float32),
    )
    shas = {}
    for ver in ("v3", "v4"):
        tmp = DveOpSpec(
            name=name,
            opcode=_dv._SUB_OPCODE_FOR_NAME[name],
            uops=lower(spec, ver=ver),
            rd1_en=True,
        )
        shas[ver] = tmp.sha(ver)
    op = _dv.DveOp(name, spec, subdim=False, uops_sha=shas)
    _dv.OPS.append(op)
    _dv.CUSTOM_DVE_SPECS[name] = spec
    _SOFTSHRINK_ADD = op
    return op


def _build(lam: tuple, rho: float):
    import concourse.tile as tile
    from concourse import bacc, mybir

    f32 = mybir.dt.float32
    f16 = mybir.dt.float16
    Alu = mybir.AluOpType
    Act = mybir.ActivationFunctionType

    ssop = _get_softshrink_add()
    nc = bacc.Bacc("TRN2", target_bir_lowering=False, debug=False, num_devices=NCORES)
    yt16_d = nc.dram_tensor("yt16", [N, BC], f16, kind="ExternalInput").ap()
    w_d = nc.dram_tensor("w", [S, KT, P, N], f16, kind="ExternalInput").ap()
    out_d = nc.dram_tensor("out", [N, BC], f32, kind="ExternalOutput").ap()

    with tile.TileContext(nc) as tc:
        with (
            tc.tile_pool(name="state", bufs=1) as state,
            tc.tile_pool(name="wpool", bufs=WPOOL_BUFS) as wpool,
            tc.tile_pool(name="scratch", bufs=SCRATCH_BUFS) as scratch,
            tc.tile_pool(name="psum", bufs=PSUM_BUFS, space="PSUM") as psum_pool,
        ):
            Y16 = [state.tile([P, BC], f16, tag=f"Y16_{i}", name=f"Y16_{i}") for i in range(KT)]
            for i in range(KT):
                nc.sync.dma_start(Y16[i][:, 0:CHUNK], yt16_d[i * P : (i + 1) * P, 0:CHUNK])
            ut = [state.tile([P, BC], f16, tag=f"u{i}", name=f"u{i}") for i in range(MT)]
            rt = [
                [state.tile([P, BC], f16, tag=f"r{p}_{i}", name=f"r{p}_{i}") for i in range(KT)]
                for p in range(2)
            ]

            for s in range(S):
                lam_s = float(lam[s])
                rhs = Y16 if s == 0 else rt[(s - 1) % 2]
                rnew = rt[s % 2]

                slabs = [wpool.tile([P, N], f16, tag="w", name=f"w{s}_{i}") for i in range(KT)]
                for kk in range(KT):
                    nc.sync.dma_start(slabs[kk][:], w_d[s, kk, :, :])
                if s == 0:
                    for i in range(KT):
                        nc.sync.dma_start(
                            Y16[i][:, CHUNK:BC], yt16_d[i * P : (i + 1) * P, CHUNK:BC]
                        )

                pending = []

                def tail_ops(unit, m, bs, t_t):
                    """e-add + rnew stt (or final z), lagged behind t/clip so
                    in-order engine queues never stall on fresh results."""
                    if s == S - 1:
                        z_t = scratch.tile([P, CHUNK], f32, tag="e", name=f"z{m}_{bs.start}")
                        zeng = nc.vector if unit % 2 == 0 else nc.gpsimd
                        zeng.tensor_tensor(z_t[:], t_t[:], ut[m][:, bs], Alu.subtract)
                        nc.sync.dma_start(out_d[m * P : (m + 1) * P, bs], z_t[:])
                        return
                    if rho == 1.0:
                        nc.vector._custom_dve(
                            ssop,
                            out=rnew[m][:, bs],
                            in0=t_t[:],
                            in1=Y16[m][:, bs],
                            s0=lam_s,
                            s1=-2.0 * rho,
                        )
                    else:
                        e_t = scratch.tile([P, CHUNK], f32, tag="e", name=f"e{s}_{m}_{bs.start}")
                        nc.vector.scalar_tensor_tensor(
                            e_t[:], t_t[:], rho, Y16[m][:, bs], Alu.mult, Alu.add
                        )
                        nc.vector.scalar_tensor_tensor(
                            rnew[m][:, bs], ut[m][:, bs], -2.0 * rho, e_t[:], Alu.mult, Alu.add
                        )

                for b in range(NB):
                    bs = slice(b * CHUNK, (b + 1) * CHUNK)
                    for m in range(MT):
                        ps = psum_pool.tile([P, CHUNK], f32, tag="ps", name=f"ps{s}_{m}_{b}")
                        for k in range(KT):
                            nc.tensor.matmul(
                                ps[:],
                                slabs[k][:, m * P : m * P + P],
                                rhs[k][:, bs],
                                start=(k == 0),
                                stop=(k == KT - 1),
                            )
                        t_t = scratch.tile([P, CHUNK], f32, tag="t", name=f"t{s}_{m}_{b}")
                        if s == 0:
                            # u == 0: t = psum
                            nc.scalar.activation(t_t[:], ps[:], Act.Copy)
                        else:
                            nc.vector.tensor_tensor(t_t[:], ps[:], ut[m][:, bs], Alu.add)
                        # clip -> u' (fp16): final stage writes uc into ut as well
                        nc.gpsimd.tensor_scalar(ut[m][:, bs], t_t[:], lam_s, -lam_s, Alu.min, Alu.max)
                        pending.append((b * MT + m, m, bs, t_t))
                        if len(pending) > (0 if s == S - 1 else LAG):
                            tail_ops(*pending.pop(0))
                for item in pending:
                    tail_ops(*item)

    nc.compile()
    return nc


def _pack_weights(M2, M3):
    W = np.transpose(M2 * M3[:, :, None], (0, 2, 1)).astype(np.float32)  # [S,N,N] lhsT
    Wp = W.reshape(S, KT, P, N).astype(np.float16)
    return np.ascontiguousarray(Wp)


def _numpy_fallback(y, M2, M3, alpha, lamb, rho):
    yT = y.T.astype(np.float32)
    z = np.zeros_like(yT)
    u = np.zeros_like(yT)
    for s in range(M2.shape[0]):
        x = M3[s][:, None] * (M2[s] @ (yT + rho * (z - u)))
        x1 = alpha[s] * x + (1.0 - alpha[s]) * z
        v = x1 + u
        t = lamb[s] / rho
        z = np.sign(v) * np.maximum(np.abs(v) - t, 0.0)
        u = v - z
    return np.ascontiguousarray(z.T)


def kernel(y, M2, M3, alpha, lamb, rho):
    global LAST_RESULT
    y = np.asarray(y, dtype=np.float32)
    M2 = np.asarray(M2, dtype=np.float32)
    M3 = np.asarray(M3, dtype=np.float32)
    alpha = np.asarray(alpha, dtype=np.float32)
    lamb = np.asarray(lamb, dtype=np.float32)
    rho_f = float(np.asarray(rho))

    if (
        not np.all(alpha == 1.0)
        or y.shape != (B, N)
        or M2.shape != (S, N, N)
        or M3.shape != (S, N)
    ):
        return _numpy_fallback(y, M2, M3, alpha, lamb, rho_f)

    from concourse.bass_utils import run_bass_kernel_spmd

    lam = tuple(float(l) / rho_f for l in lamb)
    key = (lam, rho_f, WPOOL_BUFS, SCRATCH_BUFS, FIN_BUFS, PSUM_BUFS, E_SPLIT, LAG)
    nc = _compiled.get(key)
    if nc is None:
        nc = _build(lam, rho_f)
        _compiled[key] = nc

    Wp = _pack_weights(M2, M3)
    in_maps = []
    for c in range(NCORES):
        yt_c = np.ascontiguousarray(y[c * BC : (c + 1) * BC, :].T).astype(np.float16)
        in_maps.append({"yt16": yt_c, "w": Wp})

    try:
        import antenv.axon_hooks  # noqa: F401
        trace = bool(os.environ.get("BASS_TRACE"))
    except ImportError:
        # No NTFF hook registry in this image: make sure bass_utils never
        # takes the trace path (it would crash importing antenv.axon_hooks).
        os.environ["BASS_NEVER_TRACE"] = "1"
        trace = False
    res = run_bass_kernel_spmd(nc, in_maps, core_ids=list(range(NCORES)), trace=trace)
    LAST_RESULT = res

    out = np.empty((B, N), dtype=np.float32)
    for c in range(NCORES):
        out[c * BC : (c + 1) * BC, :] = res.results[c]["out"].T
    return out
